# revision 11
# baseline (speedup 1.0000x reference)
"""Trainium2 Bass kernel for an 8-head self-attention block (MHA).

Problem: x[2, 4096, 512], 8 heads x 64 dims, torch-Linear q/k/v/o projections,
softmax attention, residual:  out = softmax(q k^T / 8) v @ Wo^T + bo + x.

Sharding (8 NeuronCores, no collectives): core c handles batch b = c // 4 and
query rows (c % 4) * 1024 ... + 1024, for ALL heads.  K/V for the full
sequence are computed on every core of a batch group (projections are cheap
relative to attention), so the output projection is fully local to a core.

Layouts are chosen so no on-device transpose is ever needed; the host passes
x^T and pre-transposed weights:
  - kT[f, s] f32 in SBUF; scores lhsT slices [64, 128]
  - qT[f, q] f32, pre-scaled by 1/sqrt(64); scores rhs slices [64, 512]
  - scores^T chunk [s=128, q=512] = kT_sl.T @ qT_sl on PE (psum)
  - exp on ACT -> P~ bf16 (no max subtraction: scores are O(1) here)
  - V bf16, s-chunk-major with per-head-pair 130-col blocks containing
    constant-1 columns, so each PV matmul also accumulates the softmax
    denominator into an adjacent psum row (even head: rows 0-63 data /
    row 64 denom; odd head: row 63 denom / rows 64-127 data)
  - normalize: DVE reciprocal of denom row, K=1 ones-outer matmul to
    broadcast it across partitions, DVE multiply -> oT[f, q] f32
  - output projection contracts oT with Wo^T; bias enters as a K=1
    outer-product matmul into the same psum; residual added on DVE.
"""

import numpy as np

B = 2
S = 4096
E = 512
H = 8
D = 64
P = 128
EC = E // P          # 4 e-chunks
FC = E // P          # 4 f-chunks
NJ = S // P          # 32 s-chunks
QR = S // 4          # 1024 query rows per core
NQS = QR // 512      # 2 query strips of 512
NKS = S // 512       # 8 s-strips of 512

_CACHE = {}


def _build_nc():
    import concourse.bass as bass
    import concourse.tile as tile
    from concourse import bacc, mybir

    f32 = mybir.dt.float32
    bf16 = mybir.dt.bfloat16
    AFT = mybir.ActivationFunctionType
    Alu = mybir.AluOpType

    nc = bacc.Bacc("TRN2", target_bir_lowering=False, debug=False, num_devices=8)

    xT_d = nc.declare_dram_parameter("xT", [E, S], f32, isOutput=False)
    xqT_d = nc.declare_dram_parameter("xqT", [E, QR], f32, isOutput=False)
    xres_d = nc.declare_dram_parameter("xres", [QR, E], f32, isOutput=False)
    wqT_d = nc.declare_dram_parameter("wqT", [E, E], f32, isOutput=False)
    wkT_d = nc.declare_dram_parameter("wkT", [E, E], f32, isOutput=False)
    wvT_d = nc.declare_dram_parameter("wvT", [E, E], f32, isOutput=False)
    woT_d = nc.declare_dram_parameter("woT", [E, E], f32, isOutput=False)
    bq_d = nc.declare_dram_parameter("bq", [P, FC], f32, isOutput=False)
    bk_d = nc.declare_dram_parameter("bk", [P, FC], f32, isOutput=False)
    bv_d = nc.declare_dram_parameter("bv", [E], f32, isOutput=False)
    bo_d = nc.declare_dram_parameter("bo", [E], f32, isOutput=False)
    ones_d = nc.declare_dram_parameter("ones", [1, P], f32, isOutput=False)
    out_d = nc.declare_dram_parameter("out", [QR, E], f32, isOutput=True)

    with tile.TileContext(nc) as tc:
        with tc.tile_pool(name="const", bufs=1) as const, \
             tc.tile_pool(name="persist", bufs=1) as persist:

            # ---- constants that live for the whole kernel ----
            wo_sb = const.tile([P, EC, E], f32)
            nc.sync.dma_start(
                out=wo_sb[:], in_=woT_d.ap().rearrange("(c p) f -> p c f", p=P))
            bq_sb = const.tile([P, FC], f32)
            bk_sb = const.tile([P, FC], f32)
            nc.sync.dma_start(out=bq_sb[:], in_=bq_d[:])
            nc.sync.dma_start(out=bk_sb[:], in_=bk_d[:])
            bv_sb = const.tile([P, E], f32)
            nc.sync.dma_start(
                out=bv_sb[:],
                in_=bass.AP(tensor=bv_d, offset=0, ap=[[0, P], [1, E]]))
            bo_sb = const.tile([1, E], f32)
            nc.sync.dma_start(
                out=bo_sb[:],
                in_=bass.AP(tensor=bo_d, offset=0, ap=[[0, 1], [1, E]]))
            # ones replicated on every partition so K=1 outer-product
            # matmuls can pick a lhsT row at any base partition
            ones_sb = const.tile([P, P], f32)
            nc.sync.dma_start(
                out=ones_sb[:],
                in_=bass.AP(tensor=ones_d, offset=0, ap=[[0, P], [1, P]]))

            # ---- persistent activations ----
            kT_sb = persist.tile([P, FC, S], f32)            # 64 KB/p
            qT_sb = persist.tile([P, FC, QR], f32)           # 16 KB/p
            v_sb = persist.tile([P, NJ, H, 65], bf16)        # 32.5 KB/p
            oT_sb = persist.tile([P, FC, QR], f32)           # 16 KB/p

            # constant-1 columns (softmax denominator trick)
            nc.vector.memset(v_sb[:, :, :, 64:65], 1.0)

            # ================= phase B: projections =================
            with tc.tile_pool(name="wpool", bufs=1) as wpool, \
                 tc.tile_pool(name="xtp", bufs=2) as xtp, \
                 tc.tile_pool(name="ps_b", bufs=3, space="PSUM") as ps_b:

                wq_sb = wpool.tile([P, EC, E], f32)
                wk_sb = wpool.tile([P, EC, E], f32)
                wv_sb = wpool.tile([P, EC, E], f32)
                for t, d in ((wq_sb, wqT_d), (wk_sb, wkT_d), (wv_sb, wvT_d)):
                    nc.sync.dma_start(
                        out=t[:], in_=d.ap().rearrange("(c p) f -> p c f", p=P))

                # B1: kT[f, s] = Wk @ x^T (+ bk), streaming xT strips
                for strip in range(NKS):
                    ssl = slice(strip * 512, (strip + 1) * 512)
                    xt = xtp.tile([P, EC, 512], f32, tag="xt")
                    for e in range(EC):
                        nc.sync.dma_start(
                            out=xt[:, e, :], in_=xT_d[e * P:(e + 1) * P, ssl])
                    for f in range(FC):
                        pk = ps_b.tile([P, 512], f32, tag="pb")
                        for e in range(EC):
                            nc.tensor.matmul(
                                pk[:], wk_sb[:, e, f * P:(f + 1) * P],
                                xt[:, e, :], start=(e == 0), stop=(e == EC - 1),
                                skip_group_check=True)
                        nc.vector.tensor_scalar_add(
                            kT_sb[:, f, ssl], pk[:], bk_sb[:, f:f + 1])

                # B2: qT[f, q] = (Wq @ xq^T + bq) / 8, from the query slice
                for qs in range(NQS):
                    qsl = slice(qs * 512, (qs + 1) * 512)
                    xq = xtp.tile([P, EC, 512], f32, tag="xt")
                    for e in range(EC):
                        nc.sync.dma_start(
                            out=xq[:, e, :], in_=xqT_d[e * P:(e + 1) * P, qsl])
                    for f in range(FC):
                        pq = ps_b.tile([P, 512], f32, tag="pb")
                        for e in range(EC):
                            nc.tensor.matmul(
                                pq[:], wq_sb[:, e, f * P:(f + 1) * P],
                                xq[:, e, :], start=(e == 0), stop=(e == EC - 1),
                                skip_group_check=True)
                        nc.vector.tensor_scalar(
                            qT_sb[:, f, qsl], pq[:], bq_sb[:, f:f + 1],
                            float(1.0 / np.sqrt(D)), Alu.add, Alu.mult)

                # B3: V (natural layout, bf16, interleaved with 1-columns)
                for j in range(NJ):
                    jsl = slice(j * P, (j + 1) * P)
                    xv = xtp.tile([P, EC, P], f32, tag="xv")
                    for e in range(EC):
                        nc.sync.dma_start(
                            out=xv[:, e, :], in_=xT_d[e * P:(e + 1) * P, jsl])
                    pv = ps_b.tile([P, E], f32, tag="pb")
                    for e in range(EC):
                        nc.tensor.matmul(
                            pv[:], xv[:, e, :], wv_sb[:, e, :],
                            start=(e == 0), stop=(e == EC - 1),
                            skip_group_check=True)
                    pv_v = pv[:].rearrange("p (h d) -> p h d", h=H)
                    bv_v = bv_sb[:].rearrange("p (h d) -> p h d", h=H)
                    nc.vector.tensor_add(v_sb[:, j, :, 0:64], pv_v[:], bv_v[:])

            # ================= phase C: attention =================
            with tc.tile_pool(name="work", bufs=3) as work, \
                 tc.tile_pool(name="rec", bufs=2) as rec, \
                 tc.tile_pool(name="ps_sc", bufs=2, space="PSUM") as ps_sc, \
                 tc.tile_pool(name="ps_pv", bufs=2, space="PSUM") as ps_pv:

                def emit_normalize(pvp, h):
                    fc = h // 2
                    even = (h % 2) == 0
                    rc = rec.tile([P, QR], f32, tag="rc", name="rc")
                    nc.vector.reciprocal(rc[64:65, :], pvp[64:65, :])
                    # broadcast 1/den down 64 partitions via K=1 matmul
                    # (reuses an sc psum slot)
                    bc = ps_sc.tile([P, QR], f32, tag="sc", name="bc")
                    for hf in range(QR // 512):
                        hsl = slice(hf * 512, (hf + 1) * 512)
                        nc.tensor.matmul(bc[0:64, hsl], ones_sb[64:65, 0:64],
                                         rc[64:65, hsl], start=True, stop=True,
                                         skip_group_check=True)
                    # DVE reads at most one PSUM operand: stage PV in SBUF
                    st = rec.tile([64, QR], f32, tag="st", name="st")
                    nc.vector.tensor_copy(st[:], pvp[0:64, :])
                    if even:
                        nc.vector.tensor_mul(oT_sb[0:64, fc, :], st[:],
                                             bc[0:64, :])
                    else:
                        # engines cannot shift partitions; multiply at base 0
                        # and DMA (which can) into rows 64-127
                        st2 = rec.tile([64, QR], f32, tag="st2", name="st2")
                        nc.vector.tensor_mul(st2[:], st[:], bc[0:64, :])
                        nc.sync.dma_start(out=oT_sb[64:128, fc, :], in_=st2[:])

                pending = None
                for h in range(H):
                    fc = h // 2
                    fr = (h % 2) * 64
                    pvp = ps_pv.tile([65, QR], f32, tag="pv", name="pvp")
                    prev_pt = None
                    for j in range(NJ):
                        # scores^T chunk [s=128, q=1024] (two 512 halves)
                        sc = ps_sc.tile([P, QR], f32, tag="sc", name="sc")
                        for hf in range(QR // 512):
                            hsl = slice(hf * 512, (hf + 1) * 512)
                            nc.tensor.matmul(
                                sc[:, hsl],
                                kT_sb[fr:fr + 64, fc, j * P:(j + 1) * P],
                                qT_sb[fr:fr + 64, fc, hsl],
                                start=True, stop=True, skip_group_check=True)
                        pt = work.tile([P, QR], bf16, tag="pt", name="pt")
                        nc.scalar.activation(pt[:], sc[:], AFT.Exp)
                        # software pipeline: PV for chunk j-1 is emitted after
                        # the scores matmuls of chunk j so PE never waits on
                        # the exp of the chunk it just produced
                        if prev_pt is not None:
                            for hf in range(QR // 512):
                                hsl = slice(hf * 512, (hf + 1) * 512)
                                nc.tensor.matmul(
                                    pvp[0:65, hsl], v_sb[:, j - 1, h, :],
                                    prev_pt[:, hsl],
                                    start=(j - 1 == 0), stop=False,
                                    skip_group_check=True)
                        if pending is not None and j == 6:
                            # deferred normalize of the previous head, mid
                            # stream so its reciprocal is long done by the
                            # time PE reaches the broadcast matmul
                            emit_normalize(*pending)
                            pending = None
                        prev_pt = pt
                    for hf in range(QR // 512):
                        hsl = slice(hf * 512, (hf + 1) * 512)
                        nc.tensor.matmul(
                            pvp[0:65, hsl], v_sb[:, NJ - 1, h, :],
                            prev_pt[:, hsl], start=False, stop=True,
                            skip_group_check=True)
                    pending = (pvp, h)
                emit_normalize(*pending)

            # ============ phase D: output projection + residual ============
            with tc.tile_pool(name="outp", bufs=2) as outp, \
                 tc.tile_pool(name="ps_o", bufs=2, space="PSUM") as ps_o:
                for qc in range(QR // P):
                    po = ps_o.tile([P, E], f32, tag="po")
                    nc.tensor.matmul(po[:], ones_sb[0:1, :], bo_sb[:],
                                     start=True, stop=False,
                                     skip_group_check=True)
                    for e in range(EC):
                        nc.tensor.matmul(
                            po[:], oT_sb[:, e, qc * P:(qc + 1) * P],
                            wo_sb[:, e, :], start=False, stop=(e == EC - 1),
                            skip_group_check=True)
                    xr = outp.tile([P, E], f32, tag="xr")
                    nc.sync.dma_start(
                        out=xr[:], in_=xres_d[qc * P:(qc + 1) * P, :])
                    ot = outp.tile([P, E], f32, tag="ot")
                    nc.vector.tensor_add(ot[:], po[:], xr[:])
                    nc.sync.dma_start(out=out_d[qc * P:(qc + 1) * P, :],
                                      in_=ot[:])

    nc.compile()
    return nc


def _get_nc():
    if "nc" not in _CACHE:
        _CACHE["nc"] = _build_nc()
    return _CACHE["nc"]


def run_spmd(in_maps, **kw):
    from concourse.bass_utils import run_bass_kernel_spmd
    nc = _get_nc()
    return run_bass_kernel_spmd(nc, in_maps, list(range(8)), **kw)


def make_in_maps(x, Wq, bq, Wk, bk, Wv, bv, Wo, bo):
    x = np.asarray(x, dtype=np.float32)
    f32c = lambda a: np.ascontiguousarray(np.asarray(a, dtype=np.float32))
    wqT = f32c(np.asarray(Wq).T)
    wkT = f32c(np.asarray(Wk).T)
    wvT = f32c(np.asarray(Wv).T)
    woT = f32c(np.asarray(Wo).T)
    bq_r = f32c(np.asarray(bq).reshape(FC, P).T)
    bk_r = f32c(np.asarray(bk).reshape(FC, P).T)
    bv_a = f32c(bv)
    bo_a = f32c(bo)
    ones = np.ones((1, P), dtype=np.float32)
    xT = [f32c(x[b].T) for b in range(B)]

    in_maps = []
    for c in range(8):
        b, r = c // 4, c % 4
        in_maps.append({
            "xT": xT[b],
            "xqT": f32c(xT[b][:, r * QR:(r + 1) * QR]),
            "xres": f32c(x[b, r * QR:(r + 1) * QR]),
            "wqT": wqT, "wkT": wkT, "wvT": wvT, "woT": woT,
            "bq": bq_r, "bk": bk_r, "bv": bv_a, "bo": bo_a,
            "ones": ones,
        })
    return in_maps


def assemble(results):
    out = np.empty((B, S, E), dtype=np.float32)
    for c in range(8):
        b, r = c // 4, c % 4
        out[b, r * QR:(r + 1) * QR] = results[c]["out"]
    return out


def kernel(x, Wq, bq, Wk, bk, Wv, bv, Wo, bo):
    in_maps = make_in_maps(x, Wq, bq, Wk, bk, Wv, bv, Wo, bo)
    res = run_spmd(in_maps)
    return assemble(res.results)


# revision 15
# speedup vs baseline: 2.4641x; 2.4641x over previous
"""Trainium2 Bass kernel for an 8-head self-attention block (MHA).

Problem: x[2, 4096, 512], 8 heads x 64 dims, torch-Linear q/k/v/o projections,
softmax attention, residual:  out = softmax(q k^T / 8) v @ Wo^T + bo + x.

Sharding (8 NeuronCores, no collectives): core c handles batch b = c // 4 and
query rows (c % 4) * 1024 ... + 1024, for ALL heads.  K/V for the full
sequence are computed on every core of a batch group (projections are cheap
relative to attention), so the output projection is fully local to a core.

Layouts are chosen so no on-device transpose is ever needed; the host passes
x^T and pre-transposed weights:
  - kT[f, s] f32 in SBUF; scores lhsT slices [64, 128]
  - qT[f, q] f32, pre-scaled by 1/sqrt(64); scores rhs slices [64, 512]
  - scores^T chunk [s=128, q=512] = kT_sl.T @ qT_sl on PE (psum)
  - exp on ACT -> P~ bf16 (no max subtraction: scores are O(1) here)
  - V bf16, s-chunk-major with per-head-pair 130-col blocks containing
    constant-1 columns, so each PV matmul also accumulates the softmax
    denominator into an adjacent psum row (even head: rows 0-63 data /
    row 64 denom; odd head: row 63 denom / rows 64-127 data)
  - normalize: DVE reciprocal of denom row, K=1 ones-outer matmul to
    broadcast it across partitions, DVE multiply -> oT[f, q] f32
  - output projection contracts oT with Wo^T; bias enters as a K=1
    outer-product matmul into the same psum; residual added on DVE.
"""

import numpy as np

B = 2
S = 4096
E = 512
H = 8
D = 64
P = 128
EC = E // P          # 4 e-chunks
FC = E // P          # 4 f-chunks
NJ = S // P          # 32 s-chunks
QR = S // 4          # 1024 query rows per core
NQS = QR // 512      # 2 query strips of 512
NKS = S // 512       # 8 s-strips of 512

_CACHE = {}


def _build_nc():
    import concourse.bass as bass
    import concourse.tile as tile
    from concourse import bacc, mybir

    f32 = mybir.dt.float32
    bf16 = mybir.dt.bfloat16
    AFT = mybir.ActivationFunctionType
    Alu = mybir.AluOpType

    nc = bacc.Bacc("TRN2", target_bir_lowering=False, debug=False, num_devices=8)

    xT_d = nc.declare_dram_parameter("xT", [E, S], bf16, isOutput=False)
    xqT_d = nc.declare_dram_parameter("xqT", [E, QR], bf16, isOutput=False)
    xres_d = nc.declare_dram_parameter("xres", [QR, E], f32, isOutput=False)
    wqT_d = nc.declare_dram_parameter("wqT", [E, E], bf16, isOutput=False)
    wkT_d = nc.declare_dram_parameter("wkT", [E, E], bf16, isOutput=False)
    wvT_d = nc.declare_dram_parameter("wvT", [E, E], bf16, isOutput=False)
    woT_d = nc.declare_dram_parameter("woT", [E, E], bf16, isOutput=False)
    bq_d = nc.declare_dram_parameter("bq", [P, FC], f32, isOutput=False)
    bk_d = nc.declare_dram_parameter("bk", [P, FC], f32, isOutput=False)
    bv_d = nc.declare_dram_parameter("bv", [E], f32, isOutput=False)
    ones_d = nc.declare_dram_parameter("ones", [1, P], f32, isOutput=False)
    out_d = nc.declare_dram_parameter("out", [QR, E], f32, isOutput=True)

    with tile.TileContext(nc) as tc:
        with tc.tile_pool(name="const", bufs=1) as const, \
             tc.tile_pool(name="persist", bufs=1) as persist:

            # ---- constants that live for the whole kernel ----
            wo_sb = const.tile([P, EC, E], bf16)
            nc.sync.dma_start(
                out=wo_sb[:], in_=woT_d.ap().rearrange("(c p) f -> p c f", p=P))
            bq_sb = const.tile([P, FC], f32)
            bk_sb = const.tile([P, FC], f32)
            nc.sync.dma_start(out=bq_sb[:], in_=bq_d[:])
            nc.sync.dma_start(out=bk_sb[:], in_=bk_d[:])
            bv_sb = const.tile([P, E], f32)
            nc.sync.dma_start(
                out=bv_sb[:],
                in_=bass.AP(tensor=bv_d, offset=0, ap=[[0, P], [1, E]]))
            # ones replicated on every partition so K=1 outer-product
            # matmuls can pick a lhsT row at any base partition
            ones_sb = const.tile([P, P], f32)
            nc.sync.dma_start(
                out=ones_sb[:],
                in_=bass.AP(tensor=ones_d, offset=0, ap=[[0, P], [1, P]]))

            # ---- persistent activations ----
            kT_sb = persist.tile([P, FC, S], bf16)           # 32 KB/p
            qT_sb = persist.tile([P, FC, QR], bf16)          # 8 KB/p
            v_sb = persist.tile([P, NJ, H, 65], bf16)        # 32.5 KB/p
            oT_sb = persist.tile([P, FC, QR], bf16)          # 8 KB/p

            # constant-1 columns (softmax denominator trick)
            nc.vector.memset(v_sb[:, :, :, 64:65], 1.0)

            # ================= phase B: projections =================
            with tc.tile_pool(name="wpool", bufs=1) as wpool, \
                 tc.tile_pool(name="xtp", bufs=2) as xtp, \
                 tc.tile_pool(name="ps_b", bufs=3, space="PSUM") as ps_b:

                wq_sb = wpool.tile([P, EC, E], bf16)
                wk_sb = wpool.tile([P, EC, E], bf16)
                wv_sb = wpool.tile([P, EC, E], bf16)
                for t, d in ((wq_sb, wqT_d), (wk_sb, wkT_d), (wv_sb, wvT_d)):
                    nc.sync.dma_start(
                        out=t[:], in_=d.ap().rearrange("(c p) f -> p c f", p=P))

                # B1: kT[f, s] = Wk @ x^T (+ bk), streaming xT strips
                for strip in range(NKS):
                    ssl = slice(strip * 512, (strip + 1) * 512)
                    xt = xtp.tile([P, EC, 512], bf16, tag="xt")
                    for e in range(EC):
                        nc.sync.dma_start(
                            out=xt[:, e, :], in_=xT_d[e * P:(e + 1) * P, ssl])
                    for f in range(FC):
                        pk = ps_b.tile([P, 512], f32, tag="pb")
                        for e in range(EC):
                            nc.tensor.matmul(
                                pk[:], wk_sb[:, e, f * P:(f + 1) * P],
                                xt[:, e, :], start=(e == 0), stop=(e == EC - 1),
                                skip_group_check=True)
                        nc.vector.tensor_scalar_add(
                            kT_sb[:, f, ssl], pk[:], bk_sb[:, f:f + 1])

                # B2: qT[f, q] = (Wq @ xq^T + bq) / 8, from the query slice
                for qs in range(NQS):
                    qsl = slice(qs * 512, (qs + 1) * 512)
                    xq = xtp.tile([P, EC, 512], bf16, tag="xt")
                    for e in range(EC):
                        nc.sync.dma_start(
                            out=xq[:, e, :], in_=xqT_d[e * P:(e + 1) * P, qsl])
                    for f in range(FC):
                        pq = ps_b.tile([P, 512], f32, tag="pb")
                        for e in range(EC):
                            nc.tensor.matmul(
                                pq[:], wq_sb[:, e, f * P:(f + 1) * P],
                                xq[:, e, :], start=(e == 0), stop=(e == EC - 1),
                                skip_group_check=True)
                        nc.vector.tensor_scalar(
                            qT_sb[:, f, qsl], pq[:], bq_sb[:, f:f + 1],
                            float(1.0 / np.sqrt(D)), Alu.add, Alu.mult)

                # B3: V (natural layout, bf16, interleaved with 1-columns)
                for j in range(NJ):
                    jsl = slice(j * P, (j + 1) * P)
                    xv = xtp.tile([P, EC, P], bf16, tag="xv")
                    for e in range(EC):
                        nc.sync.dma_start(
                            out=xv[:, e, :], in_=xT_d[e * P:(e + 1) * P, jsl])
                    pv = ps_b.tile([P, E], f32, tag="pb")
                    for e in range(EC):
                        nc.tensor.matmul(
                            pv[:], xv[:, e, :], wv_sb[:, e, :],
                            start=(e == 0), stop=(e == EC - 1),
                            skip_group_check=True)
                    pv_v = pv[:].rearrange("p (h d) -> p h d", h=H)
                    bv_v = bv_sb[:].rearrange("p (h d) -> p h d", h=H)
                    nc.vector.tensor_add(v_sb[:, j, :, 0:64], pv_v[:], bv_v[:])

            # ================= phase C: attention =================
            with tc.tile_pool(name="work", bufs=3) as work, \
                 tc.tile_pool(name="rec", bufs=2) as rec, \
                 tc.tile_pool(name="ps_sc", bufs=2, space="PSUM") as ps_sc, \
                 tc.tile_pool(name="ps_pv", bufs=2, space="PSUM") as ps_pv:

                def emit_normalize(pvp, h):
                    fc = h // 2
                    even = (h % 2) == 0
                    rc = rec.tile([P, QR], f32, tag="rc", name="rc")
                    nc.vector.reciprocal(rc[64:65, :], pvp[64:65, :])
                    # broadcast 1/den down 64 partitions via K=1 matmul
                    # (reuses an sc psum slot)
                    bc = ps_sc.tile([P, QR], f32, tag="sc", name="bc")
                    for hf in range(QR // 512):
                        hsl = slice(hf * 512, (hf + 1) * 512)
                        nc.tensor.matmul(bc[0:64, hsl], ones_sb[64:65, 0:64],
                                         rc[64:65, hsl], start=True, stop=True,
                                         skip_group_check=True)
                    # DVE reads at most one PSUM operand: stage PV in SBUF
                    st = rec.tile([64, QR], f32, tag="st", name="st")
                    nc.vector.tensor_copy(st[:], pvp[0:64, :])
                    if even:
                        nc.vector.tensor_mul(oT_sb[0:64, fc, :], st[:],
                                             bc[0:64, :])
                    else:
                        # engines cannot shift partitions; multiply at base 0
                        # and DMA (which can) into rows 64-127
                        st2 = rec.tile([64, QR], bf16, tag="st2", name="st2")
                        nc.vector.tensor_mul(st2[:], st[:], bc[0:64, :])
                        nc.sync.dma_start(out=oT_sb[64:128, fc, :], in_=st2[:])

                pending = None
                for h in range(H):
                    fc = h // 2
                    fr = (h % 2) * 64
                    pvp = ps_pv.tile([65, QR], f32, tag="pv", name="pvp")
                    prev_pt = None
                    for j in range(NJ):
                        # scores^T chunk [s=128, q=1024] (two 512 halves)
                        sc = ps_sc.tile([P, QR], f32, tag="sc", name="sc")
                        for hf in range(QR // 512):
                            hsl = slice(hf * 512, (hf + 1) * 512)
                            nc.tensor.matmul(
                                sc[:, hsl],
                                kT_sb[fr:fr + 64, fc, j * P:(j + 1) * P],
                                qT_sb[fr:fr + 64, fc, hsl],
                                start=True, stop=True, skip_group_check=True)
                        pt = work.tile([P, QR], bf16, tag="pt", name="pt")
                        nc.scalar.activation(pt[:], sc[:], AFT.Exp)
                        # software pipeline: PV for chunk j-1 is emitted after
                        # the scores matmuls of chunk j so PE never waits on
                        # the exp of the chunk it just produced
                        if prev_pt is not None:
                            for hf in range(QR // 512):
                                hsl = slice(hf * 512, (hf + 1) * 512)
                                nc.tensor.matmul(
                                    pvp[0:65, hsl], v_sb[:, j - 1, h, :],
                                    prev_pt[:, hsl],
                                    start=(j - 1 == 0), stop=False,
                                    skip_group_check=True)
                        if pending is not None and j == 6:
                            # deferred normalize of the previous head, mid
                            # stream so its reciprocal is long done by the
                            # time PE reaches the broadcast matmul
                            emit_normalize(*pending)
                            pending = None
                        prev_pt = pt
                    for hf in range(QR // 512):
                        hsl = slice(hf * 512, (hf + 1) * 512)
                        nc.tensor.matmul(
                            pvp[0:65, hsl], v_sb[:, NJ - 1, h, :],
                            prev_pt[:, hsl], start=False, stop=True,
                            skip_group_check=True)
                    pending = (pvp, h)
                emit_normalize(*pending)

            # ============ phase D: output projection + residual ============
            with tc.tile_pool(name="outp", bufs=2) as outp, \
                 tc.tile_pool(name="ps_o", bufs=2, space="PSUM") as ps_o:
                for qc in range(QR // P):
                    po = ps_o.tile([P, E], f32, tag="po")
                    for e in range(EC):
                        nc.tensor.matmul(
                            po[:], oT_sb[:, e, qc * P:(qc + 1) * P],
                            wo_sb[:, e, :], start=(e == 0), stop=(e == EC - 1),
                            skip_group_check=True)
                    xr = outp.tile([P, E], f32, tag="xr")
                    nc.sync.dma_start(
                        out=xr[:], in_=xres_d[qc * P:(qc + 1) * P, :])
                    ot = outp.tile([P, E], f32, tag="ot")
                    nc.vector.tensor_add(ot[:], po[:], xr[:])
                    nc.sync.dma_start(out=out_d[qc * P:(qc + 1) * P, :],
                                      in_=ot[:])

    nc.compile()
    return nc


def _get_nc():
    if "nc" not in _CACHE:
        _CACHE["nc"] = _build_nc()
    return _CACHE["nc"]


def run_spmd(in_maps, **kw):
    from concourse.bass_utils import run_bass_kernel_spmd
    nc = _get_nc()
    return run_bass_kernel_spmd(nc, in_maps, list(range(8)), **kw)


def make_in_maps(x, Wq, bq, Wk, bk, Wv, bv, Wo, bo):
    import ml_dtypes
    bf = ml_dtypes.bfloat16
    x = np.asarray(x, dtype=np.float32)
    f32c = lambda a: np.ascontiguousarray(np.asarray(a, dtype=np.float32))
    bfc = lambda a: np.ascontiguousarray(
        np.asarray(a, dtype=np.float32).astype(bf))
    wqT = bfc(np.asarray(Wq).T)
    wkT = bfc(np.asarray(Wk).T)
    wvT = bfc(np.asarray(Wv).T)
    woT = bfc(np.asarray(Wo).T)
    bq_r = f32c(np.asarray(bq).reshape(FC, P).T)
    bk_r = f32c(np.asarray(bk).reshape(FC, P).T)
    bv_a = f32c(bv)
    bo_a = np.asarray(bo, dtype=np.float32)
    ones = np.ones((1, P), dtype=np.float32)
    xT = [bfc(x[b].T) for b in range(B)]

    in_maps = []
    for c in range(8):
        b, r = c // 4, c % 4
        in_maps.append({
            "xT": xT[b],
            "xqT": np.ascontiguousarray(xT[b][:, r * QR:(r + 1) * QR]),
            # output bias folded into the residual tile (host-side, free)
            "xres": f32c(x[b, r * QR:(r + 1) * QR] + bo_a),
            "wqT": wqT, "wkT": wkT, "wvT": wvT, "woT": woT,
            "bq": bq_r, "bk": bk_r, "bv": bv_a,
            "ones": ones,
        })
    return in_maps


def assemble(results):
    out = np.empty((B, S, E), dtype=np.float32)
    for c in range(8):
        b, r = c // 4, c % 4
        out[b, r * QR:(r + 1) * QR] = results[c]["out"]
    return out


def kernel(x, Wq, bq, Wk, bk, Wv, bv, Wo, bo):
    in_maps = make_in_maps(x, Wq, bq, Wk, bk, Wv, bv, Wo, bo)
    res = run_spmd(in_maps)
    return assemble(res.results)


# revision 18
# speedup vs baseline: 3.0186x; 1.2251x over previous
"""Trainium2 Bass kernel for an 8-head self-attention block (MHA).

Problem: x[2, 4096, 512], 8 heads x 64 dims, torch-Linear q/k/v/o projections,
softmax attention, residual:  out = softmax(q k^T / 8) v @ Wo^T + bo + x.

Sharding (8 NeuronCores, no collectives): core c handles batch b = c // 4 and
query rows (c % 4) * 1024 ... + 1024, for ALL heads.  K/V for the full
sequence are computed on every core of a batch group (projections are cheap
relative to attention), so the output projection is fully local to a core.

Layouts are chosen so no on-device transpose is ever needed; the host passes
x^T and pre-transposed weights:
  - kT[f, s] f32 in SBUF; scores lhsT slices [64, 128]
  - qT[f, q] f32, pre-scaled by 1/sqrt(64); scores rhs slices [64, 512]
  - scores^T chunk [s=128, q=512] = kT_sl.T @ qT_sl on PE (psum)
  - exp on ACT -> P~ bf16 (no max subtraction: scores are O(1) here)
  - V bf16, s-chunk-major with per-head-pair 130-col blocks containing
    constant-1 columns, so each PV matmul also accumulates the softmax
    denominator into an adjacent psum row (even head: rows 0-63 data /
    row 64 denom; odd head: row 63 denom / rows 64-127 data)
  - normalize: DVE reciprocal of denom row, K=1 ones-outer matmul to
    broadcast it across partitions, DVE multiply -> oT[f, q] f32
  - output projection contracts oT with Wo^T; bias enters as a K=1
    outer-product matmul into the same psum; residual added on DVE.
"""

import numpy as np

B = 2
S = 4096
E = 512
H = 8
D = 64
P = 128
EC = E // P          # 4 e-chunks
FC = E // P          # 4 f-chunks
NJ = S // P          # 32 s-chunks
QR = S // 4          # 1024 query rows per core
NQS = QR // 512      # 2 query strips of 512
NKS = S // 512       # 8 s-strips of 512

_CACHE = {}


def _build_nc():
    import concourse.bass as bass
    import concourse.tile as tile
    from concourse import bacc, mybir

    f32 = mybir.dt.float32
    bf16 = mybir.dt.bfloat16
    AFT = mybir.ActivationFunctionType
    Alu = mybir.AluOpType

    nc = bacc.Bacc("TRN2", target_bir_lowering=False, debug=False, num_devices=8)

    xT_d = nc.declare_dram_parameter("xT", [E, S], bf16, isOutput=False)
    xqT_d = nc.declare_dram_parameter("xqT", [E, QR], bf16, isOutput=False)
    xres_d = nc.declare_dram_parameter("xres", [QR, E], f32, isOutput=False)
    wqT_d = nc.declare_dram_parameter("wqT", [E, E], bf16, isOutput=False)
    wkT_d = nc.declare_dram_parameter("wkT", [E, E], bf16, isOutput=False)
    wvT_d = nc.declare_dram_parameter("wvT", [E, E], bf16, isOutput=False)
    woT_d = nc.declare_dram_parameter("woT", [E, E], bf16, isOutput=False)
    bq_d = nc.declare_dram_parameter("bq", [P, FC], f32, isOutput=False)
    bk_d = nc.declare_dram_parameter("bk", [P, FC], f32, isOutput=False)
    bv_d = nc.declare_dram_parameter("bv", [E], f32, isOutput=False)
    ident_d = nc.declare_dram_parameter("ident", [P, P], bf16, isOutput=False)
    out_d = nc.declare_dram_parameter("out", [QR, E], f32, isOutput=True)

    with tile.TileContext(nc) as tc:
        with tc.tile_pool(name="const", bufs=1) as const, \
             tc.tile_pool(name="persist", bufs=1) as persist:

            # ---- constants that live for the whole kernel ----
            wo_sb = const.tile([P, EC, E], bf16)
            nc.sync.dma_start(
                out=wo_sb[:], in_=woT_d.ap().rearrange("(c p) f -> p c f", p=P))
            bq_sb = const.tile([P, FC], f32)
            bk_sb = const.tile([P, FC], f32)
            nc.sync.dma_start(out=bq_sb[:], in_=bq_d[:])
            nc.sync.dma_start(out=bk_sb[:], in_=bk_d[:])
            bv_sb = const.tile([P, E], f32)
            nc.sync.dma_start(
                out=bv_sb[:],
                in_=bass.AP(tensor=bv_d, offset=0, ap=[[0, P], [1, E]]))
            # identity for PE transposes
            ident_sb = const.tile([P, P], bf16)
            nc.sync.dma_start(out=ident_sb[:], in_=ident_d[:])

            # ---- persistent activations ----
            kT_sb = persist.tile([P, FC, S], bf16)           # 32 KB/p
            qT_sb = persist.tile([P, FC, QR], bf16)          # 8 KB/p
            v_sb = persist.tile([P, NJ, H, 65], bf16)        # 32.5 KB/p
            oT_sb = persist.tile([P, FC, QR], bf16)          # 8 KB/p

            # constant-1 columns (softmax denominator trick)
            nc.vector.memset(v_sb[:, :, :, 64:65], 1.0)

            # ================= phase B: projections =================
            with tc.tile_pool(name="wpool", bufs=1) as wpool, \
                 tc.tile_pool(name="xtp", bufs=2) as xtp, \
                 tc.tile_pool(name="ps_b", bufs=3, space="PSUM") as ps_b:

                wq_sb = wpool.tile([P, EC, E], bf16)
                wk_sb = wpool.tile([P, EC, E], bf16)
                wv_sb = wpool.tile([P, EC, E], bf16)
                for t, d in ((wq_sb, wqT_d), (wk_sb, wkT_d), (wv_sb, wvT_d)):
                    nc.sync.dma_start(
                        out=t[:], in_=d.ap().rearrange("(c p) f -> p c f", p=P))

                # B1: kT[f, s] = Wk @ x^T (+ bk), streaming xT strips
                for strip in range(NKS):
                    ssl = slice(strip * 512, (strip + 1) * 512)
                    xt = xtp.tile([P, EC, 512], bf16, tag="xt")
                    for e in range(EC):
                        nc.sync.dma_start(
                            out=xt[:, e, :], in_=xT_d[e * P:(e + 1) * P, ssl])
                    for f in range(FC):
                        pk = ps_b.tile([P, 512], f32, tag="pb")
                        for e in range(EC):
                            nc.tensor.matmul(
                                pk[:], wk_sb[:, e, f * P:(f + 1) * P],
                                xt[:, e, :], start=(e == 0), stop=(e == EC - 1),
                                skip_group_check=True)
                        nc.vector.tensor_scalar_add(
                            kT_sb[:, f, ssl], pk[:], bk_sb[:, f:f + 1])

                # B2: qT[f, q] = (Wq @ xq^T + bq) / 8, from the query slice
                for qs in range(NQS):
                    qsl = slice(qs * 512, (qs + 1) * 512)
                    xq = xtp.tile([P, EC, 512], bf16, tag="xt")
                    for e in range(EC):
                        nc.sync.dma_start(
                            out=xq[:, e, :], in_=xqT_d[e * P:(e + 1) * P, qsl])
                    for f in range(FC):
                        pq = ps_b.tile([P, 512], f32, tag="pb")
                        for e in range(EC):
                            nc.tensor.matmul(
                                pq[:], wq_sb[:, e, f * P:(f + 1) * P],
                                xq[:, e, :], start=(e == 0), stop=(e == EC - 1),
                                skip_group_check=True)
                        nc.vector.tensor_scalar(
                            qT_sb[:, f, qsl], pq[:], bq_sb[:, f:f + 1],
                            float(1.0 / np.sqrt(D)), Alu.add, Alu.mult)

                # B3: V (natural layout, bf16, interleaved with 1-columns)
                for j in range(NJ):
                    jsl = slice(j * P, (j + 1) * P)
                    xv = xtp.tile([P, EC, P], bf16, tag="xv")
                    for e in range(EC):
                        nc.sync.dma_start(
                            out=xv[:, e, :], in_=xT_d[e * P:(e + 1) * P, jsl])
                    pv = ps_b.tile([P, E], f32, tag="pb")
                    for e in range(EC):
                        nc.tensor.matmul(
                            pv[:], xv[:, e, :], wv_sb[:, e, :],
                            start=(e == 0), stop=(e == EC - 1),
                            skip_group_check=True)
                    pv_v = pv[:].rearrange("p (h d) -> p h d", h=H)
                    bv_v = bv_sb[:].rearrange("p (h d) -> p h d", h=H)
                    nc.vector.tensor_add(v_sb[:, j, :, 0:64], pv_v[:], bv_v[:])

            # ================= phase C: attention =================
            # PV runs in "natural" orientation: lhsT = P~ slice [s=128,
            # q=128], rhs = [V | 1] [s=128, 65] -> psum o[q, 65].  That
            # streams 65 columns per (qchunk, j) instead of 1024, puts the
            # softmax denominator in a psum COLUMN (so normalization is a
            # cheap per-partition tensor_scalar), and the small o tiles are
            # PE-transposed into the oT layout phase D needs.
            NQC = QR // P  # 8 query chunks of 128
            with tc.tile_pool(name="work", bufs=3) as work, \
                 tc.tile_pool(name="opool", bufs=2) as opool, \
                 tc.tile_pool(name="ps_sc", bufs=2, space="PSUM") as ps_sc, \
                 tc.tile_pool(name="ps_pv", bufs=2, space="PSUM") as ps_pv:

                def emit_normalize(pvp, h):
                    fc = h // 2
                    fr = (h % 2) * 64
                    rcp = opool.tile([P, NQC, 1], f32, tag="rcp", name="rcp")
                    nc.vector.reciprocal(rcp[:], pvp[:, :, 64:65])
                    o_sb = opool.tile([P, NQC, 64], bf16, tag="o", name="o_sb")
                    for qc in range(NQC):
                        nc.vector.tensor_scalar_mul(
                            o_sb[:, qc, :], pvp[:, qc, 0:64], rcp[:, qc, :])
                    for qc in range(NQC):
                        # transpose [128 q, 64 d] -> [64 d, 128 q] on PE,
                        # directly at the head's partition base
                        tp = ps_sc.tile([P, P], bf16, tag="sc", name="tp")
                        nc.tensor.transpose(tp[fr:fr + 64, :], o_sb[:, qc, :],
                                            ident_sb[:])
                        nc.vector.tensor_copy(
                            oT_sb[fr:fr + 64, fc, qc * P:(qc + 1) * P],
                            tp[fr:fr + 64, :])

                pending = None
                for h in range(H):
                    fc = h // 2
                    fr = (h % 2) * 64
                    pvp = ps_pv.tile([P, NQC, P], f32, tag="pv", name="pvp")
                    prev_pt = None
                    for j in range(NJ):
                        # scores^T chunk [s=128, q=1024] (two 512 halves)
                        sc = ps_sc.tile([P, QR], f32, tag="sc", name="sc")
                        for hf in range(QR // 512):
                            hsl = slice(hf * 512, (hf + 1) * 512)
                            nc.tensor.matmul(
                                sc[:, hsl],
                                kT_sb[fr:fr + 64, fc, j * P:(j + 1) * P],
                                qT_sb[fr:fr + 64, fc, hsl],
                                start=True, stop=True, skip_group_check=True)
                        pt = work.tile([P, QR], bf16, tag="pt", name="pt")
                        nc.scalar.activation(pt[:], sc[:], AFT.Exp)
                        # software pipeline: PV for chunk j-1 is emitted after
                        # the scores matmuls of chunk j so PE never waits on
                        # the exp of the chunk it just produced
                        if prev_pt is not None:
                            for qc in range(NQC):
                                nc.tensor.matmul(
                                    pvp[:, qc, 0:65],
                                    prev_pt[:, qc * P:(qc + 1) * P],
                                    v_sb[:, j - 1, h, :],
                                    start=(j - 1 == 0), stop=False,
                                    skip_group_check=True)
                        if pending is not None and j == 6:
                            # deferred normalize of the previous head, mid
                            # stream so its DVE work is long done by the time
                            # PE reaches the transposes
                            emit_normalize(*pending)
                            pending = None
                        prev_pt = pt
                    for qc in range(NQC):
                        nc.tensor.matmul(
                            pvp[:, qc, 0:65], prev_pt[:, qc * P:(qc + 1) * P],
                            v_sb[:, NJ - 1, h, :], start=False, stop=True,
                            skip_group_check=True)
                    pending = (pvp, h)
                emit_normalize(*pending)

            # ============ phase D: output projection + residual ============
            with tc.tile_pool(name="outp", bufs=2) as outp, \
                 tc.tile_pool(name="ps_o", bufs=2, space="PSUM") as ps_o:
                for qc in range(QR // P):
                    po = ps_o.tile([P, E], f32, tag="po")
                    for e in range(EC):
                        nc.tensor.matmul(
                            po[:], oT_sb[:, e, qc * P:(qc + 1) * P],
                            wo_sb[:, e, :], start=(e == 0), stop=(e == EC - 1),
                            skip_group_check=True)
                    xr = outp.tile([P, E], f32, tag="xr")
                    nc.sync.dma_start(
                        out=xr[:], in_=xres_d[qc * P:(qc + 1) * P, :])
                    ot = outp.tile([P, E], f32, tag="ot")
                    nc.vector.tensor_add(ot[:], po[:], xr[:])
                    nc.sync.dma_start(out=out_d[qc * P:(qc + 1) * P, :],
                                      in_=ot[:])

    nc.compile()
    return nc


def _get_nc():
    if "nc" not in _CACHE:
        _CACHE["nc"] = _build_nc()
    return _CACHE["nc"]


def run_spmd(in_maps, **kw):
    from concourse.bass_utils import run_bass_kernel_spmd
    nc = _get_nc()
    return run_bass_kernel_spmd(nc, in_maps, list(range(8)), **kw)


def make_in_maps(x, Wq, bq, Wk, bk, Wv, bv, Wo, bo):
    import ml_dtypes
    bf = ml_dtypes.bfloat16
    x = np.asarray(x, dtype=np.float32)
    f32c = lambda a: np.ascontiguousarray(np.asarray(a, dtype=np.float32))
    bfc = lambda a: np.ascontiguousarray(
        np.asarray(a, dtype=np.float32).astype(bf))
    wqT = bfc(np.asarray(Wq).T)
    wkT = bfc(np.asarray(Wk).T)
    wvT = bfc(np.asarray(Wv).T)
    woT = bfc(np.asarray(Wo).T)
    bq_r = f32c(np.asarray(bq).reshape(FC, P).T)
    bk_r = f32c(np.asarray(bk).reshape(FC, P).T)
    bv_a = f32c(bv)
    bo_a = np.asarray(bo, dtype=np.float32)
    ident = np.eye(P, dtype=np.float32).astype(bf)
    xT = [bfc(x[b].T) for b in range(B)]

    in_maps = []
    for c in range(8):
        b, r = c // 4, c % 4
        in_maps.append({
            "xT": xT[b],
            "xqT": np.ascontiguousarray(xT[b][:, r * QR:(r + 1) * QR]),
            # output bias folded into the residual tile (host-side, free)
            "xres": f32c(x[b, r * QR:(r + 1) * QR] + bo_a),
            "wqT": wqT, "wkT": wkT, "wvT": wvT, "woT": woT,
            "bq": bq_r, "bk": bk_r, "bv": bv_a,
            "ident": ident,
        })
    return in_maps


def assemble(results):
    out = np.empty((B, S, E), dtype=np.float32)
    for c in range(8):
        b, r = c // 4, c % 4
        out[b, r * QR:(r + 1) * QR] = results[c]["out"]
    return out


def kernel(x, Wq, bq, Wk, bk, Wv, bv, Wo, bo):
    in_maps = make_in_maps(x, Wq, bq, Wk, bk, Wv, bv, Wo, bo)
    res = run_spmd(in_maps)
    return assemble(res.results)
